# revision 1
# baseline (speedup 1.0000x reference)
"""Trainium2 Bass kernel for nn_ConditionalDLFactorized17 (moe_routing).

Math (reference):
    logits = einsum('tbc,ec->tbe', x, assign_w) + assign_b      # router
    resp   = softmax(logits, -1)
    importance = resp.sum over tokens;  loss = .01*std(imp,ddof=1)/mean(imp)
    y = einsum('tbe,eoi,tbi->tbo', resp, pw_w1.reshape(e,o,i), x) + pw_B

Strategy: data-parallel over tokens (T*B = 16384 -> 2048/core on 8 cores),
pw_w1 replicated.  Per core, per 128-token tile:
    H_e = x_tile @ W_e^T  (f32r matmuls, K=512, accumulated in PSUM)
    y_tile += resp[:, e] * H_e   (DVE scalar_tensor_tensor, fused mul-add)
Host pre-transposes x and pw_w1 so no on-device transposes are needed
(PE contraction runs over the partition dim).  The scalar importance
reduction is finished on host from per-core partials.
"""
import threading

import numpy as np

import concourse.bass as bass
import concourse.mybir as mybir
import concourse.tile as tile
from concourse import bacc
from concourse.bass_utils import run_bass_kernel_spmd

F32 = mybir.dt.float32
F32R = mybir.dt.float32r

T, B, C, OUT, NE = 2048, 8, 512, 512, 16
NCORES = 8
TOKENS = T * B                  # 16384
TOK_CORE = TOKENS // NCORES     # 2048
P = 128
NTILE = TOK_CORE // P           # 16
CB = C // P                     # 4 contraction blocks
LOSS_SCALE = 0.01

_lock = threading.Lock()
_cache = {}


def _build():
    from contextlib import ExitStack

    nc = bacc.Bacc()
    xT = nc.dram_tensor("xT", [CB, P, TOK_CORE], F32R, kind="ExternalInput")
    w2 = nc.dram_tensor("w2", [NE, CB, P, OUT], F32R, kind="ExternalInput")
    awT = nc.dram_tensor("awT", [CB, P, NE], F32R, kind="ExternalInput")
    ab_rep = nc.dram_tensor("ab_rep", [P, NE], F32, kind="ExternalInput")
    pwB_rep = nc.dram_tensor("pwB_rep", [P, OUT], F32, kind="ExternalInput")
    y = nc.dram_tensor("y", [TOK_CORE, OUT], F32, kind="ExternalOutput")
    imp = nc.dram_tensor("imp", [1, NE], F32, kind="ExternalOutput")

    with tile.TileContext(nc) as tc, ExitStack() as ctx:
        const = ctx.enter_context(tc.tile_pool(name="const", bufs=1))
        ypool = ctx.enter_context(tc.tile_pool(name="ypool", bufs=2))
        spool = ctx.enter_context(tc.tile_pool(name="spool", bufs=3))
        ps_h = ctx.enter_context(tc.tile_pool(name="ps_h", bufs=5, space="PSUM"))
        ps_l = ctx.enter_context(tc.tile_pool(name="ps_l", bufs=2, space="PSUM"))
        ps_i = ctx.enter_context(tc.tile_pool(name="ps_i", bufs=1, space="PSUM"))

        # ---- resident tensors ----
        xt_sb = const.tile([P, CB, TOK_CORE], F32R, tag="xt")
        nc.sync.dma_start(xt_sb[:], xT.ap().rearrange("cb p t -> p cb t"))
        w2_sb = const.tile([P, NE, CB, OUT], F32R, tag="w2")
        for e in range(NE):
            nc.sync.dma_start(
                w2_sb[:, e], w2.ap()[e].rearrange("cb p o -> p cb o")
            )
        awT_sb = const.tile([P, CB, NE], F32R, tag="awT")
        nc.sync.dma_start(awT_sb[:], awT.ap().rearrange("cb p e -> p cb e"))
        ab_sb = const.tile([P, NE], F32, tag="ab")
        nc.sync.dma_start(ab_sb[:], ab_rep.ap())
        pwB_sb = const.tile([P, OUT], F32, tag="pwB")
        nc.sync.dma_start(pwB_sb[:], pwB_rep.ap())
        ones_sb = const.tile([P, 1], F32, tag="ones")
        nc.vector.memset(ones_sb[:], 1.0)
        racc_sb = const.tile([P, NE], F32, tag="racc")
        nc.vector.memset(racc_sb[:], 0.0)

        for t in range(NTILE):
            xt = xt_sb[:, :, t * P : (t + 1) * P]  # [128, CB, 128]
            # ---- router: logits[tok, e] ----
            lg_ps = ps_l.tile([P, NE], F32, tag="lg")
            for cb in range(CB):
                nc.tensor.matmul(
                    lg_ps[:], xt[:, cb, :], awT_sb[:, cb, :],
                    start=(cb == 0), stop=(cb == CB - 1),
                )
            lg = spool.tile([P, NE], F32, tag="lgsb")
            nc.vector.tensor_add(lg[:], lg_ps[:], ab_sb[:])
            # ---- softmax over e (free dim) ----
            mx = spool.tile([P, 1], F32, tag="mx")
            nc.vector.reduce_max(mx[:], lg[:], axis=mybir.AxisListType.X)
            negm = spool.tile([P, 1], F32, tag="negm")
            nc.vector.tensor_scalar_mul(negm[:], mx[:], -1.0)
            expt = spool.tile([P, NE], F32, tag="expt")
            ssum = spool.tile([P, 1], F32, tag="ssum")
            nc.scalar.activation(
                expt[:], lg[:], mybir.ActivationFunctionType.Exp,
                bias=negm[:], scale=1.0, accum_out=ssum[:],
            )
            rinv = spool.tile([P, 1], F32, tag="rinv")
            nc.vector.reciprocal(rinv[:], ssum[:])
            resp = spool.tile([P, NE], F32, tag="resp")
            nc.vector.tensor_scalar_mul(resp[:], expt[:], rinv[:])
            # importance partial accumulation (over tiles) in SBUF
            nc.vector.tensor_add(racc_sb[:], racc_sb[:], resp[:])

            # ---- main: y_tile = sum_e resp[:,e] * (x_tile @ W_e^T) + pw_B ----
            yt = ypool.tile([P, OUT], F32, tag="yt")
            nc.vector.tensor_copy(yt[:], pwB_sb[:])
            for e in range(NE):
                h_ps = ps_h.tile([P, OUT], F32, tag="h")
                for cb in range(CB):
                    nc.tensor.matmul(
                        h_ps[:], xt[:, cb, :], w2_sb[:, e, cb, :],
                        start=(cb == 0), stop=(cb == CB - 1),
                    )
                nc.vector.scalar_tensor_tensor(
                    yt[:], h_ps[:], resp[:, e : e + 1], yt[:],
                    op0=mybir.AluOpType.mult, op1=mybir.AluOpType.add,
                )
            nc.sync.dma_start(y.ap()[t * P : (t + 1) * P, :], yt[:])

        # ---- importance: contract racc over the 128 partitions ----
        imp_ps = ps_i.tile([1, NE], F32, tag="imp")
        nc.tensor.matmul(imp_ps[:], ones_sb[:], racc_sb[:], start=True, stop=True)
        imp_sb = spool.tile([1, NE], F32, tag="impsb")
        nc.vector.tensor_copy(imp_sb[:], imp_ps[:])
        nc.sync.dma_start(imp.ap(), imp_sb[:])

    nc.compile()
    return nc


def _get_nc():
    with _lock:
        if "nc" not in _cache:
            _cache["nc"] = _build()
        return _cache["nc"]


def kernel(x, assign_w, assign_b, pw_w1, pw_B, _want_results=False):
    x = np.asarray(x, dtype=np.float32)
    assign_w = np.asarray(assign_w, dtype=np.float32)
    assign_b = np.asarray(assign_b, dtype=np.float32)
    pw_w1 = np.asarray(pw_w1, dtype=np.float32)
    pw_B = np.asarray(pw_B, dtype=np.float32)

    nc = _get_nc()

    # ---- host-side sharding / layout prep ----
    xt_full = x.reshape(TOKENS, C)
    w2_host = np.ascontiguousarray(
        pw_w1.reshape(NE, OUT, C).transpose(0, 2, 1)
    ).reshape(NE, CB, P, OUT)                      # [e, i, o] blocked over i
    awT_host = np.ascontiguousarray(assign_w.T).reshape(CB, P, NE)
    ab_host = np.ascontiguousarray(np.tile(assign_b.reshape(1, NE), (P, 1)))
    pwB_host = np.ascontiguousarray(np.tile(pw_B.reshape(1, OUT), (P, 1)))

    in_maps = []
    for k in range(NCORES):
        shard = xt_full[k * TOK_CORE : (k + 1) * TOK_CORE]          # [2048, 512]
        xT_host = np.ascontiguousarray(shard.T).reshape(CB, P, TOK_CORE)
        in_maps.append(
            {
                "xT": xT_host,
                "w2": w2_host,
                "awT": awT_host,
                "ab_rep": ab_host,
                "pwB_rep": pwB_host,
            }
        )

    res = run_bass_kernel_spmd(nc, in_maps, core_ids=list(range(NCORES)))

    y = np.concatenate([r["y"] for r in res.results], axis=0).reshape(T, B, OUT)
    importance = np.sum(
        np.stack([r["imp"][0] for r in res.results]).astype(np.float64), axis=0
    )
    loss = np.float32(
        LOSS_SCALE * np.std(importance, ddof=1) / np.mean(importance)
    )
    if _want_results:
        return (y, loss), res
    return y, loss


# revision 3
# speedup vs baseline: 1.1132x; 1.1132x over previous
"""Trainium2 Bass kernel for nn_ConditionalDLFactorized17 (moe_routing).

Math (reference):
    logits = einsum('tbc,ec->tbe', x, assign_w) + assign_b      # router
    resp   = softmax(logits, -1)
    importance = resp.sum over tokens;  loss = .01*std(imp,ddof=1)/mean(imp)
    y = einsum('tbe,eoi,tbi->tbo', resp, pw_w1.reshape(e,o,i), x) + pw_B

Strategy: data-parallel over tokens (T*B = 16384 -> 2048/core on 8 cores),
pw_w1 replicated.  Per core, per 128-token tile:
    H_e = x_tile @ W_e^T  (f32r matmuls, K=512, accumulated in PSUM)
    y_tile += resp[:, e] * H_e   (DVE scalar_tensor_tensor, fused mul-add)
Host pre-transposes x and pw_w1 so no on-device transposes are needed
(PE contraction runs over the partition dim).  The scalar importance
reduction is finished on host from per-core partials.
"""
import threading

import numpy as np

import concourse.bass as bass
import concourse.mybir as mybir
import concourse.tile as tile
from concourse import bacc
from concourse.bass_utils import run_bass_kernel_spmd

F32 = mybir.dt.float32
F32R = mybir.dt.float32r

T, B, C, OUT, NE = 2048, 8, 512, 512, 16
NCORES = 8
TOKENS = T * B                  # 16384
TOK_CORE = TOKENS // NCORES     # 2048
P = 128
NTILE = TOK_CORE // P           # 16
CB = C // P                     # 4 contraction blocks
LOSS_SCALE = 0.01

_lock = threading.Lock()
_cache = {}


def _build():
    from contextlib import ExitStack

    nc = bacc.Bacc()
    xT = nc.dram_tensor("xT", [CB, P, TOK_CORE], F32R, kind="ExternalInput")
    w2 = nc.dram_tensor("w2", [NE, CB, P, OUT], F32R, kind="ExternalInput")
    awT = nc.dram_tensor("awT", [CB, P, NE], F32R, kind="ExternalInput")
    ab_rep = nc.dram_tensor("ab_rep", [P, NE], F32, kind="ExternalInput")
    pwB_rep = nc.dram_tensor("pwB_rep", [P, OUT], F32, kind="ExternalInput")
    y = nc.dram_tensor("y", [TOK_CORE, OUT], F32, kind="ExternalOutput")
    imp = nc.dram_tensor("imp", [1, NE], F32, kind="ExternalOutput")

    with tile.TileContext(nc) as tc, ExitStack() as ctx:
        const = ctx.enter_context(tc.tile_pool(name="const", bufs=1))
        ypool = ctx.enter_context(tc.tile_pool(name="ypool", bufs=1))
        spool = ctx.enter_context(tc.tile_pool(name="spool", bufs=3))
        ps_h = ctx.enter_context(tc.tile_pool(name="ps_h", bufs=5, space="PSUM"))
        ps_l = ctx.enter_context(tc.tile_pool(name="ps_l", bufs=2, space="PSUM"))
        ps_i = ctx.enter_context(tc.tile_pool(name="ps_i", bufs=1, space="PSUM"))

        # ---- resident tensors ----
        # xT loaded per contraction block so the router can start early.
        xt_sb = const.tile([P, CB, TOK_CORE], F32R, tag="xt")
        awT_sb = const.tile([P, CB, NE], F32R, tag="awT")
        nc.sync.dma_start(awT_sb[:], awT.ap().rearrange("cb p e -> p cb e"))
        ab_sb = const.tile([P, NE], F32, tag="ab")
        nc.sync.dma_start(ab_sb[:], ab_rep.ap())
        pwB_sb = const.tile([P, OUT], F32, tag="pwB")
        nc.sync.dma_start(pwB_sb[:], pwB_rep.ap())
        for cb in range(CB):
            nc.sync.dma_start(xt_sb[:, cb, :], xT.ap()[cb])
        # weight stream: per-expert chunks land while the e-loop consumes them
        w2_sb = const.tile([P, NE, CB, OUT], F32R, tag="w2")
        for e in range(NE):
            nc.sync.dma_start(
                w2_sb[:, e], w2.ap()[e].rearrange("cb p o -> p cb o")
            )
        ones_sb = const.tile([P, 1], F32, tag="ones")
        nc.vector.memset(ones_sb[:], 1.0)
        racc_sb = const.tile([P, NE], F32, tag="racc")
        nc.vector.memset(racc_sb[:], 0.0)

        # ---- router + softmax for all tiles (overlaps the w2 stream) ----
        resps = []
        yts = []
        for t in range(NTILE):
            xt = xt_sb[:, :, t * P : (t + 1) * P]  # [128, CB, 128]
            lg_ps = ps_l.tile([P, NE], F32, tag="lg")
            for cb in range(CB):
                nc.tensor.matmul(
                    lg_ps[:], xt[:, cb, :], awT_sb[:, cb, :],
                    start=(cb == 0), stop=(cb == CB - 1),
                )
            lg = spool.tile([P, NE], F32, tag="lgsb")
            nc.vector.tensor_add(lg[:], lg_ps[:], ab_sb[:])
            mx = spool.tile([P, 1], F32, tag="mx")
            nc.vector.reduce_max(mx[:], lg[:], axis=mybir.AxisListType.X)
            negm = spool.tile([P, 1], F32, tag="negm")
            nc.vector.tensor_scalar_mul(negm[:], mx[:], -1.0)
            expt = spool.tile([P, NE], F32, tag="expt")
            ssum = spool.tile([P, 1], F32, tag="ssum")
            nc.scalar.activation(
                expt[:], lg[:], mybir.ActivationFunctionType.Exp,
                bias=negm[:], scale=1.0, accum_out=ssum[:],
            )
            rinv = spool.tile([P, 1], F32, tag="rinv")
            nc.vector.reciprocal(rinv[:], ssum[:])
            resp = spool.tile([P, NE], F32, tag=f"resp{t}")
            nc.vector.tensor_scalar_mul(resp[:], expt[:], rinv[:])
            nc.vector.tensor_add(racc_sb[:], racc_sb[:], resp[:])
            resps.append(resp)
            # y accumulator initialized with the (broadcast) pw_B bias
            yt = ypool.tile([P, OUT], F32, tag=f"yt{t}")
            nc.vector.tensor_copy(yt[:], pwB_sb[:])
            yts.append(yt)

        # ---- main loop, expert-outer: consume w2 chunks as they land ----
        for e in range(NE):
            for t in range(NTILE):
                xt = xt_sb[:, :, t * P : (t + 1) * P]
                h_ps = ps_h.tile([P, OUT], F32, tag="h")
                for cb in range(CB):
                    nc.tensor.matmul(
                        h_ps[:], xt[:, cb, :], w2_sb[:, e, cb, :],
                        start=(cb == 0), stop=(cb == CB - 1),
                    )
                nc.vector.scalar_tensor_tensor(
                    yts[t][:], h_ps[:], resps[t][:, e : e + 1], yts[t][:],
                    op0=mybir.AluOpType.mult, op1=mybir.AluOpType.add,
                )
                if e == NE - 1:
                    nc.sync.dma_start(
                        y.ap()[t * P : (t + 1) * P, :], yts[t][:]
                    )

        # ---- importance: contract racc over the 128 partitions ----
        imp_ps = ps_i.tile([1, NE], F32, tag="imp")
        nc.tensor.matmul(imp_ps[:], ones_sb[:], racc_sb[:], start=True, stop=True)
        imp_sb = spool.tile([1, NE], F32, tag="impsb")
        nc.vector.tensor_copy(imp_sb[:], imp_ps[:])
        nc.sync.dma_start(imp.ap(), imp_sb[:])

    nc.compile()
    return nc


def _get_nc():
    with _lock:
        if "nc" not in _cache:
            _cache["nc"] = _build()
        return _cache["nc"]


def kernel(x, assign_w, assign_b, pw_w1, pw_B, _want_results=False):
    x = np.asarray(x, dtype=np.float32)
    assign_w = np.asarray(assign_w, dtype=np.float32)
    assign_b = np.asarray(assign_b, dtype=np.float32)
    pw_w1 = np.asarray(pw_w1, dtype=np.float32)
    pw_B = np.asarray(pw_B, dtype=np.float32)

    nc = _get_nc()

    # ---- host-side sharding / layout prep ----
    xt_full = x.reshape(TOKENS, C)
    w2_host = np.ascontiguousarray(
        pw_w1.reshape(NE, OUT, C).transpose(0, 2, 1)
    ).reshape(NE, CB, P, OUT)                      # [e, i, o] blocked over i
    awT_host = np.ascontiguousarray(assign_w.T).reshape(CB, P, NE)
    ab_host = np.ascontiguousarray(np.tile(assign_b.reshape(1, NE), (P, 1)))
    pwB_host = np.ascontiguousarray(np.tile(pw_B.reshape(1, OUT), (P, 1)))

    in_maps = []
    for k in range(NCORES):
        shard = xt_full[k * TOK_CORE : (k + 1) * TOK_CORE]          # [2048, 512]
        xT_host = np.ascontiguousarray(shard.T).reshape(CB, P, TOK_CORE)
        in_maps.append(
            {
                "xT": xT_host,
                "w2": w2_host,
                "awT": awT_host,
                "ab_rep": ab_host,
                "pwB_rep": pwB_host,
            }
        )

    res = run_bass_kernel_spmd(nc, in_maps, core_ids=list(range(NCORES)))

    y = np.concatenate([r["y"] for r in res.results], axis=0).reshape(T, B, OUT)
    importance = np.sum(
        np.stack([r["imp"][0] for r in res.results]).astype(np.float64), axis=0
    )
    loss = np.float32(
        LOSS_SCALE * np.std(importance, ddof=1) / np.mean(importance)
    )
    if _want_results:
        return (y, loss), res
    return y, loss


# revision 9
# speedup vs baseline: 1.1211x; 1.0071x over previous
"""Trainium2 Bass kernel for nn_ConditionalDLFactorized17 (moe_routing).

Math (reference):
    logits = einsum('tbc,ec->tbe', x, assign_w) + assign_b      # router
    resp   = softmax(logits, -1)
    importance = resp.sum over tokens;  loss = .01*std(imp,ddof=1)/mean(imp)
    y = einsum('tbe,eoi,tbi->tbo', resp, pw_w1.reshape(e,o,i), x) + pw_B

Strategy: data-parallel over tokens (T*B = 16384 -> 2048/core on 8 cores),
pw_w1 replicated.  Per core, per 128-token tile:
    H_e = x_tile @ W_e^T  (f32r matmuls, K=512, accumulated in PSUM)
    y_tile += resp[:, e] * H_e   (DVE scalar_tensor_tensor, fused mul-add)
Host pre-transposes x and pw_w1 so no on-device transposes are needed
(PE contraction runs over the partition dim).  The scalar importance
reduction is finished on host from per-core partials.
"""
import threading

import numpy as np

import concourse.bass as bass
import concourse.mybir as mybir
import concourse.tile as tile
from concourse import bacc
from concourse.bass_utils import run_bass_kernel_spmd

F32 = mybir.dt.float32
F32R = mybir.dt.float32r

T, B, C, OUT, NE = 2048, 8, 512, 512, 16
NCORES = 8
TOKENS = T * B                  # 16384
TOK_CORE = TOKENS // NCORES     # 2048
P = 128
NTILE = TOK_CORE // P           # 16
CB = C // P                     # 4 contraction blocks
LOSS_SCALE = 0.01

_lock = threading.Lock()
_cache = {}


def _build():
    from contextlib import ExitStack

    nc = bacc.Bacc()
    xT = nc.dram_tensor("xT", [NTILE, CB, P, P], F32R, kind="ExternalInput")
    w2 = nc.dram_tensor("w2", [NE, CB, P, OUT], F32R, kind="ExternalInput")
    awT = nc.dram_tensor("awT", [CB, P, NE], F32R, kind="ExternalInput")
    ab_rep = nc.dram_tensor("ab_rep", [P, NE], F32, kind="ExternalInput")
    pwB_rep = nc.dram_tensor("pwB_rep", [P, OUT], F32, kind="ExternalInput")
    y = nc.dram_tensor("y", [TOK_CORE, OUT], F32, kind="ExternalOutput")
    imp = nc.dram_tensor("imp", [1, NE], F32, kind="ExternalOutput")

    with tile.TileContext(nc) as tc, ExitStack() as ctx:
        const = ctx.enter_context(tc.tile_pool(name="const", bufs=1))
        ypool = ctx.enter_context(tc.tile_pool(name="ypool", bufs=1))
        spool = ctx.enter_context(tc.tile_pool(name="spool", bufs=3))
        ps_h = ctx.enter_context(tc.tile_pool(name="ps_h", bufs=5, space="PSUM"))
        ps_l = ctx.enter_context(tc.tile_pool(name="ps_l", bufs=2, space="PSUM"))
        ps_i = ctx.enter_context(tc.tile_pool(name="ps_i", bufs=1, space="PSUM"))

        # ---- resident tensors ----
        # DMA issue order is the priority order: router inputs + first
        # x/weight chunks first so compute starts ASAP; the rest streams in
        # behind while the e-loop consumes chunk by chunk.
        xt_sb = const.tile([P, NTILE, CB, P], F32R, tag="xt")
        awT_sb = const.tile([P, CB, NE], F32R, tag="awT")
        nc.sync.dma_start(awT_sb[:], awT.ap().rearrange("cb p e -> p cb e"))
        ab_sb = const.tile([P, NE], F32, tag="ab")
        nc.sync.dma_start(ab_sb[:], ab_rep.ap())
        pwB_sb = const.tile([P, OUT], F32, tag="pwB")
        nc.sync.dma_start(pwB_sb[:], pwB_rep.ap())
        w2_sb = const.tile([P, NE, CB, OUT], F32R, tag="w2")

        def load_xt(t):
            nc.sync.dma_start(
                xt_sb[:, t], xT.ap()[t].rearrange("cb p k -> p cb k")
            )

        def load_w2(e):
            nc.sync.dma_start(
                w2_sb[:, e], w2.ap()[e].rearrange("cb p o -> p cb o")
            )

        load_xt(0)
        load_w2(0)
        for t in range(1, NTILE):
            load_xt(t)
        for e in range(1, NE):
            load_w2(e)
        ones_sb = const.tile([P, 1], F32, tag="ones")
        nc.vector.memset(ones_sb[:], 1.0)
        racc_sb = const.tile([P, NE], F32, tag="racc")
        nc.vector.memset(racc_sb[:], 0.0)

        # ---- router + softmax for all tiles (overlaps the w2 stream) ----
        resps = []
        yts = []
        for t in range(NTILE):
            xt = xt_sb[:, t]  # [128, CB, 128]
            lg_ps = ps_l.tile([P, NE], F32, tag="lg")
            for cb in range(CB):
                nc.tensor.matmul(
                    lg_ps[:], xt[:, cb, :], awT_sb[:, cb, :],
                    start=(cb == 0), stop=(cb == CB - 1),
                )
            lg = spool.tile([P, NE], F32, tag="lgsb")
            nc.vector.tensor_add(lg[:], lg_ps[:], ab_sb[:])
            mx = spool.tile([P, 1], F32, tag="mx")
            nc.vector.reduce_max(mx[:], lg[:], axis=mybir.AxisListType.X)
            negm = spool.tile([P, 1], F32, tag="negm")
            nc.vector.tensor_scalar_mul(negm[:], mx[:], -1.0)
            expt = spool.tile([P, NE], F32, tag="expt")
            ssum = spool.tile([P, 1], F32, tag="ssum")
            nc.scalar.activation(
                expt[:], lg[:], mybir.ActivationFunctionType.Exp,
                bias=negm[:], scale=1.0, accum_out=ssum[:],
            )
            rinv = spool.tile([P, 1], F32, tag="rinv")
            nc.vector.reciprocal(rinv[:], ssum[:])
            resp = spool.tile([P, NE], F32, tag=f"resp{t}")
            nc.vector.tensor_scalar_mul(resp[:], expt[:], rinv[:])
            nc.vector.tensor_add(racc_sb[:], racc_sb[:], resp[:])
            resps.append(resp)
            # y accumulator initialized with the (broadcast) pw_B bias
            yt = ypool.tile([P, OUT], F32, tag=f"yt{t}")
            nc.vector.tensor_copy(yt[:], pwB_sb[:])
            yts.append(yt)

        # ---- importance: contract racc over the 128 partitions ----
        # (done right after the router phase so it hides under the e-loop)
        imp_ps = ps_i.tile([1, NE], F32, tag="imp")
        nc.tensor.matmul(imp_ps[:], ones_sb[:], racc_sb[:], start=True, stop=True)
        imp_sb = spool.tile([1, NE], F32, tag="impsb")
        nc.vector.tensor_copy(imp_sb[:], imp_ps[:])
        nc.sync.dma_start(imp.ap(), imp_sb[:])

        # ---- main loop, expert-outer: consume w2 chunks as they land ----
        for e in range(NE):
            for t in range(NTILE):
                xt = xt_sb[:, t]
                h_ps = ps_h.tile([P, OUT], F32, tag="h")
                for cb in range(CB):
                    nc.tensor.matmul(
                        h_ps[:], xt[:, cb, :], w2_sb[:, e, cb, :],
                        start=(cb == 0), stop=(cb == CB - 1),
                    )
                nc.vector.scalar_tensor_tensor(
                    yts[t][:], h_ps[:], resps[t][:, e : e + 1], yts[t][:],
                    op0=mybir.AluOpType.mult, op1=mybir.AluOpType.add,
                )
                if e == NE - 1:
                    nc.sync.dma_start(
                        y.ap()[t * P : (t + 1) * P, :], yts[t][:]
                    )

    nc.compile()
    return nc


def _get_nc():
    with _lock:
        if "nc" not in _cache:
            _cache["nc"] = _build()
        return _cache["nc"]


def kernel(x, assign_w, assign_b, pw_w1, pw_B, _want_results=False):
    x = np.asarray(x, dtype=np.float32)
    assign_w = np.asarray(assign_w, dtype=np.float32)
    assign_b = np.asarray(assign_b, dtype=np.float32)
    pw_w1 = np.asarray(pw_w1, dtype=np.float32)
    pw_B = np.asarray(pw_B, dtype=np.float32)

    nc = _get_nc()

    # ---- host-side sharding / layout prep ----
    xt_full = x.reshape(TOKENS, C)
    w2_host = np.ascontiguousarray(
        pw_w1.reshape(NE, OUT, C).transpose(0, 2, 1)
    ).reshape(NE, CB, P, OUT)                      # [e, i, o] blocked over i
    awT_host = np.ascontiguousarray(assign_w.T).reshape(CB, P, NE)
    ab_host = np.ascontiguousarray(np.tile(assign_b.reshape(1, NE), (P, 1)))
    pwB_host = np.ascontiguousarray(np.tile(pw_B.reshape(1, OUT), (P, 1)))

    in_maps = []
    for k in range(NCORES):
        shard = xt_full[k * TOK_CORE : (k + 1) * TOK_CORE]          # [2048, 512]
        # per-tile transposed blocks: [NTILE, CB, ci, k] = x[t*128+k, cb*128+ci]
        xT_host = np.ascontiguousarray(
            shard.reshape(NTILE, P, CB, P).transpose(0, 2, 3, 1)
        )
        in_maps.append(
            {
                "xT": xT_host,
                "w2": w2_host,
                "awT": awT_host,
                "ab_rep": ab_host,
                "pwB_rep": pwB_host,
            }
        )

    res = run_bass_kernel_spmd(nc, in_maps, core_ids=list(range(NCORES)))

    y = np.concatenate([r["y"] for r in res.results], axis=0).reshape(T, B, OUT)
    importance = np.sum(
        np.stack([r["imp"][0] for r in res.results]).astype(np.float64), axis=0
    )
    loss = np.float32(
        LOSS_SCALE * np.std(importance, ddof=1) / np.mean(importance)
    )
    if _want_results:
        return (y, loss), res
    return y, loss
